# revision 33
# baseline (speedup 1.0000x reference)
"""AttLIF Trainium2 kernel (8-core data-parallel SPMD).

Reference computation (per batch shard):
  x = data @ W.T + b                       # Linear [B,T,I]->[B,T,H]
  s = mean_h(x); a = sigmoid(relu(s@w1.T+b1)@w2.T+b2)   # TA gate [B,T]
  x = x * a[:, :, None]
  LIF over T: u = a*u + x_t; sp = (u>=VTH); u *= (1-sp) # hard reset

Strategy:
  - Shard B=128 over 8 cores (16 each); W replicated.
  - Linear: single fp16 matmul pass (d_hi @ W_hi, fp32 PSUM accum).
    Measured spike error ~1.6% L2 vs the 2% gate (fp32 floor 0.05%);
    the fp8 correction pass used previously cost a 2nd full tensor pass.
  - s computed on-device as data.T @ mean_h(W) (+mean(b)); TA MLP on 16
    partitions; sigmoid gate broadcast to 128 partitions and fused into
    the PSUM drain (one scalar_tensor_tensor per hc chunk).
  - LIF: x stored [128part, t, hc, b] so each timestep is one contiguous
    [128,128] slice; membrane update+reset on DVE (2 ops/step, one
    full-width chain per token half), the spike compare runs on the
    Scalar engine (single Sign(u-VTH) per step pair) off the critical
    path, writing fp8 {-1,0,1} into per-16-step slab buffers (separate
    tiles so the outgoing DMA never blocks the chain). Spike stores go
    out every 16 steps via the GpSimd DMA ring.
  - Spikes written to DRAM as fp8 in device layout; host clamps -1 -> 0,
    converts to fp32 and transposes back.
All host-side work is layout/weight preprocessing only (transposes,
precision casts, column means of W); every data-dependent FLOP runs on
device.
"""

import functools
import numpy as np

ALPHA = 0.3
VTH = 0.3
B, T, I, H = 128, 64, 2048, 2048
NCORES = 8
BL = B // NCORES          # local batch = 16
TOK = BL * T              # 1024 tokens per core
IC = I // 128             # 16 contraction chunks
HC = H // 128             # 16 hidden chunks
NTOKC = 2                 # token chunks of 512 (8 local batches each)
TOKC = TOK // NTOKC       # 512
BC = BL // NTOKC          # 8 batches per token chunk
TDMA = 16                 # spike store granularity along t


def _dts():
    import ml_dtypes
    return np.float16, ml_dtypes.float8_e4m3


@functools.cache
def _build():
    import sys
    if "/opt/trn_rl_repo" not in sys.path:
        sys.path.insert(0, "/opt/trn_rl_repo")
    from contextlib import ExitStack
    from concourse import bacc, mybir, tile

    f32 = mybir.dt.float32
    f16 = mybir.dt.float16
    f8 = mybir.dt.float8e4
    Alu = mybir.AluOpType
    Act = mybir.ActivationFunctionType

    nc = bacc.Bacc("TRN2", target_bir_lowering=False, debug=False)

    dat_d = nc.dram_tensor("dat", [I, TOK], f16, kind="ExternalInput")
    wt_d = nc.dram_tensor("wt", [I, H], f16, kind="ExternalInput")
    bias_d = nc.dram_tensor("bias", [128, HC], f32, kind="ExternalInput")
    wbar_d = nc.dram_tensor("wbar", [128, IC], f16, kind="ExternalInput")
    bbar_d = nc.dram_tensor("bbar", [1, 1], f32, kind="ExternalInput")
    w1r_d = nc.dram_tensor("w1r", [BL, 4, T], f32, kind="ExternalInput")
    b1r_d = nc.dram_tensor("b1r", [BL, 4], f32, kind="ExternalInput")
    w2r_d = nc.dram_tensor("w2r", [BL, T, 4], f32, kind="ExternalInput")
    b2r_d = nc.dram_tensor("b2r", [BL, T], f32, kind="ExternalInput")
    spk_d = nc.dram_tensor("spk", [NTOKC, 128, T, HC, BC], f8, kind="ExternalOutput")

    s_dram = [nc.dram_tensor(f"s_scratch{i}", [TOKC], f32) for i in range(NTOKC)]
    a_dram = [nc.dram_tensor(f"a_scratch{i}", [T, BC], f32) for i in range(NTOKC)]

    with ExitStack() as ctx:
        tc = ctx.enter_context(tile.TileContext(nc))
        const = ctx.enter_context(tc.tile_pool(name="const", bufs=1))
        wpool = ctx.enter_context(tc.tile_pool(name="wpool", bufs=3))
        xpool = ctx.enter_context(tc.tile_pool(name="xpool", bufs=2))
        spool = ctx.enter_context(tc.tile_pool(name="spool", bufs=2))
        upool = ctx.enter_context(tc.tile_pool(name="upool", bufs=1))
        psum = ctx.enter_context(tc.tile_pool(name="psum", bufs=7, space="PSUM"))
        psum_s = ctx.enter_context(tc.tile_pool(name="psum_s", bufs=1, space="PSUM"))

        # ---- persistent loads (data on the ACT HWDGE ring, W on Sync) ----
        dat_sb = const.tile([128, IC, TOK], f16, tag="dat")
        datv = dat_d.ap().rearrange("(ic p) tok -> p ic tok", p=128)

        def emit_data_half(tci):
            sl = slice(tci * TOKC, (tci + 1) * TOKC)
            for icc in range(0, IC, 4):
                nc.scalar.dma_start(
                    out=dat_sb[:, icc : icc + 4, sl], in_=datv[:, icc : icc + 4, sl]
                )

        emit_data_half(0)
        wbar_sb = const.tile([128, IC], f16, tag="wbar")
        nc.sync.dma_start(out=wbar_sb, in_=wbar_d.ap())
        bias_sb = const.tile([128, HC], f32, tag="bias")
        nc.sync.dma_start(out=bias_sb, in_=bias_d.ap())
        bbar_sb = const.tile([1, 1], f32, tag="bbar")
        nc.sync.dma_start(out=bbar_sb, in_=bbar_d.ap())
        w1r_sb = const.tile([BL, 4, T], f32, tag="w1r")
        nc.sync.dma_start(out=w1r_sb, in_=w1r_d.ap())
        b1r_sb = const.tile([BL, 4], f32, tag="b1r")
        nc.sync.dma_start(out=b1r_sb, in_=b1r_d.ap())
        w2r_sb = const.tile([BL, T, 4], f32, tag="w2r")
        nc.sync.dma_start(out=w2r_sb, in_=w2r_d.ap())
        b2r_sb = const.tile([BL, T], f32, tag="b2r")
        nc.sync.dma_start(out=b2r_sb, in_=b2r_d.ap())
        nvth_sb = const.tile([128, 1], f32, tag="nvth")
        nc.vector.memset(nvth_sb, -VTH)

        # ---- per token-half: squeeze s, TA MLP, gate broadcast ----
        # tokens are t-major within each chunk: token = t*BC + b
        a_rep = const.tile([128, NTOKC, T, BC], f32, tag="a_rep")

        def emit_gate_half(tc_i):
            ps = psum_s.tile([1, TOKC], f32, tag="ps_s", name=f"ps_s{tc_i}")
            for ic in range(IC):
                nc.tensor.matmul(
                    ps,
                    lhsT=wbar_sb[:, ic : ic + 1],
                    rhs=dat_sb[:, ic, tc_i * TOKC : (tc_i + 1) * TOKC],
                    start=(ic == 0),
                    stop=(ic == IC - 1),
                )
            s_sb = const.tile([1, TOKC], f32, tag=f"s{tc_i}", name=f"s{tc_i}")
            nc.vector.tensor_scalar(
                out=s_sb, in0=ps, scalar1=bbar_sb, scalar2=None, op0=Alu.add,
            )
            # bounce through DRAM to re-partition [1,512] -> [8,64]
            nc.scalar.dma_start(out=s_dram[tc_i].ap(), in_=s_sb)
            sT_sb = const.tile([BC, T], f32, tag=f"sT{tc_i}", name=f"sT{tc_i}")
            nc.scalar.dma_start(
                out=sT_sb, in_=s_dram[tc_i].ap().rearrange("(t b) -> b t", b=BC)
            )

            h1_sb = const.tile([BC, 4], f32, tag=f"h1_{tc_i}", name=f"h1_{tc_i}")
            tmp_sb = const.tile([BC, T], f32, tag=f"ta_tmp{tc_i}", name=f"ta_tmp{tc_i}")
            for r in range(4):
                nc.vector.tensor_tensor(
                    out=tmp_sb, in0=sT_sb, in1=w1r_sb[:BC, r : r + 1, :], op=Alu.mult
                )
                nc.vector.tensor_reduce(
                    out=h1_sb[:, r : r + 1], in_=tmp_sb,
                    axis=mybir.AxisListType.X, op=Alu.add,
                )
            nc.vector.tensor_tensor(out=h1_sb, in0=h1_sb, in1=b1r_sb[:BC], op=Alu.add)
            h1c_sb = const.tile([BC, 4], f32, tag=f"h1c{tc_i}", name=f"h1c{tc_i}")
            nc.scalar.activation(out=h1c_sb, in_=h1_sb, func=Act.Relu)
            acc = [
                const.tile([BC, T], f32, tag=f"acc{tc_i}_{r}", name=f"acc{tc_i}_{r}")
                for r in range(4)
            ]
            nc.vector.scalar_tensor_tensor(
                out=acc[0], in0=w2r_sb[:BC, :, 0:1], scalar=h1c_sb[:, 0:1],
                in1=b2r_sb[:BC], op0=Alu.mult, op1=Alu.add,
            )
            for r in range(1, 4):
                nc.vector.scalar_tensor_tensor(
                    out=acc[r], in0=w2r_sb[:BC, :, r : r + 1], scalar=h1c_sb[:, r : r + 1],
                    in1=acc[r - 1], op0=Alu.mult, op1=Alu.add,
                )
            # sigmoid into rows 0:8 of a zeroed [32, T] pad tile, then 32x32
            # block-transposes -> aT [T, BC] (t on partitions), bounce through
            # DRAM to broadcast as [128, T, BC]
            a16p_sb = const.tile([32, T], f32, tag=f"a16p{tc_i}", name=f"a16p{tc_i}")
            aTp_sb = const.tile([T, 32], f32, tag=f"aTp{tc_i}", name=f"aTp{tc_i}")
            nc.vector.memset(a16p_sb, 0.0)
            nc.scalar.activation(out=a16p_sb[:BC, :], in_=acc[3], func=Act.Sigmoid)
            for blk in range(2):
                nc.vector.transpose(
                    out=aTp_sb[32 * blk : 32 * blk + 32, :],
                    in_=a16p_sb[:, 32 * blk : 32 * blk + 32],
                )
            nc.scalar.dma_start(
                out=a_dram[tc_i].ap(), in_=aTp_sb[:, :BC]
            )
            nc.scalar.dma_start(
                out=a_rep[:, tc_i : tc_i + 1],
                in_=a_dram[tc_i].ap().unsqueeze(0).to_broadcast((128, T, BC)),
            )

        emit_gate_half(0)
        emit_data_half(1)

        # ---- LIF emitter: sliced hc range so the last half's first chain
        # can interleave with the remaining drains ----
        u_a = upool.tile([128, HC, BC], f32, tag="u_a")
        ubb = [
            upool.tile([128, 2, HC, BC], f32, tag=f"ubb{i}", name=f"ubb{i}")
            for i in range(4)
        ]

        def emit_lif(tc_i, x_sb, spk_sb, lo, hi, t_start, t_end):
            if t_start == 0:
                nc.vector.memset(u_a[:, lo:hi, :], 0.0)
            for t in range(t_start, t_end):
                x_t = x_sb[:, t, lo:hi, :]
                u_b = ubb[(t // 2) % 4][:, t % 2, lo:hi, :]
                nc.vector.scalar_tensor_tensor(
                    out=u_b, in0=u_a[:, lo:hi, :], scalar=ALPHA, in1=x_t,
                    op0=Alu.mult, op1=Alu.add,
                )
                if t % 2 == 1:
                    pair = ubb[(t // 2) % 4][:, :, lo:hi, :]
                    # Sign(u - VTH) in {-1, 0, 1}; host clamps -1 -> 0
                    nc.scalar.activation(
                        out=spk_sb[t // TDMA][:, (t - 1) % TDMA : (t - 1) % TDMA + 2, lo:hi, :],
                        in_=pair, func=Act.Sign, bias=nvth_sb,
                    )
                if t + 1 < t_end:
                    # the final reset is dead work: u is never read again
                    # (the next chain re-memsets u_a)
                    nc.vector.scalar_tensor_tensor(
                        out=u_a[:, lo:hi, :], in0=u_b, scalar=VTH, in1=u_b,
                        op0=Alu.is_lt, op1=Alu.mult,
                    )
                if tc_i == NTOKC - 1 and t >= T - TDMA and t % 4 == 3:
                    # final slab of the last chain: four quarter stores on the
                    # idle scalar HWDGE ring so the critical last transfer is
                    # small and starts as early as possible
                    q0 = (t - (T - TDMA)) // 4 * 4
                    nc.scalar.dma_start(
                        out=spk_d.ap()[
                            tc_i : tc_i + 1, :,
                            T - TDMA + q0 : T - TDMA + q0 + 4, lo:hi, :
                        ],
                        in_=spk_sb[(T - 1) // TDMA][:, q0 : q0 + 4, lo:hi, :],
                    )
                elif t % TDMA == TDMA - 1:
                    nc.gpsimd.dma_start(
                        out=spk_d.ap()[tc_i : tc_i + 1, :, t - TDMA + 1 : t + 1, lo:hi, :],
                        in_=spk_sb[t // TDMA][:, :, lo:hi, :],
                    )

        # ---- main fp16 matmul + gate drain + LIF ----
        for tc_i in range(NTOKC):
            t0 = tc_i * TOKC
            x_sb = xpool.tile([128, T, HC, BC], f32, tag="x")
            spk_sb = [
                spool.tile([128, TDMA, HC, BC], f8, tag=f"spk{k}",
                           name=f"spk{tc_i}_{k}")
                for k in range(T // TDMA)
            ]
            for hcq in range(HC // 4):
                wsls = []
                for hcp_i in range(2):
                    h0 = (hcq * 2 + hcp_i) * 256
                    wsl = wpool.tile([128, IC, 256], f16, tag="wsl", name=f"wsl{hcp_i}")
                    nc.sync.dma_start(
                        out=wsl, in_=wt_d[:, h0 : h0 + 256].rearrange("(ic p) h -> p ic h", p=128)
                    )
                    wsls.append(wsl)

                def emit_main(hcp_i, sub):
                    hc = hcq * 4 + hcp_i * 2 + sub
                    ps = psum.tile([128, TOKC], f32, tag="ps_mm", name=f"ps_{hc}")
                    for ic in range(IC):
                        nc.tensor.matmul(
                            ps,
                            lhsT=wsls[hcp_i][:, ic, sub * 128 : sub * 128 + 128],
                            rhs=dat_sb[:, ic, t0 : t0 + TOKC],
                            start=(ic == 0),
                            stop=(ic == IC - 1),
                        )
                    return ps

                def emit_drain(hcp_i, sub, ps):
                    hc = hcq * 4 + hcp_i * 2 + sub
                    # add bias, gate (DVE) straight from PSUM; all APs are
                    # (t, b)-ordered so reads/writes are contiguous
                    nc.vector.scalar_tensor_tensor(
                        out=x_sb[:, :, hc : hc + 1, :],
                        in0=ps,
                        scalar=bias_sb[:, hc : hc + 1],
                        in1=a_rep[:, tc_i : tc_i + 1],
                        op0=Alu.add, op1=Alu.mult,
                    )

                for hcp_i in range(2):
                    for sub in range(2):
                        ps = emit_main(hcp_i, sub)
                        emit_drain(hcp_i, sub, ps)
            if tc_i + 1 < NTOKC:
                emit_gate_half(tc_i + 1)
            emit_lif(tc_i, x_sb, spk_sb, 0, HC, 0, T)

    nc.compile()
    return nc


def _host_prep(data, W, b, w1, b1, w2, b2):
    f16, f8 = _dts()
    data = np.ascontiguousarray(data, dtype=np.float32)
    W = np.ascontiguousarray(W, dtype=np.float32)

    Wh = W.astype(f16)
    wt = np.ascontiguousarray(Wh.T)                     # [I, H] fp16
    bias = np.ascontiguousarray(b.reshape(HC, 128).T, dtype=np.float32)
    wbar = W.mean(axis=0, dtype=np.float64).astype(np.float32)  # [I]
    wbar_t = np.ascontiguousarray(wbar.reshape(IC, 128).T).astype(f16)
    bbar = np.array([[b.mean(dtype=np.float64)]], dtype=np.float32)
    w1r = np.ascontiguousarray(np.broadcast_to(w1[None], (BL, 4, T)), dtype=np.float32)
    b1r = np.ascontiguousarray(np.broadcast_to(b1[None], (BL, 4)), dtype=np.float32)
    w2r = np.ascontiguousarray(np.broadcast_to(w2[None], (BL, T, 4)), dtype=np.float32)
    b2r = np.ascontiguousarray(np.broadcast_to(b2[None], (BL, T)), dtype=np.float32)

    in_maps = []
    for c in range(NCORES):
        # tokens t-major within each 512-token chunk: token = t*BC + b
        dc = np.ascontiguousarray(
            data[c * BL : (c + 1) * BL]
            .reshape(NTOKC, BC, T, I)
            .transpose(0, 2, 1, 3)
            .reshape(TOK, I)
            .T
        )                                               # [I, TOK] fp32
        dh = dc.astype(f16)
        in_maps.append({
            "dat": dh, "wt": wt,
            "bias": bias, "wbar": wbar_t, "bbar": bbar,
            "w1r": w1r, "b1r": b1r, "w2r": w2r, "b2r": b2r,
        })
    return in_maps


def _gather(results):
    outs = []
    for c in range(NCORES):
        # spikes are Sign(u - VTH) in {-1, 0, 1}; clamp negatives to 0
        spk = np.maximum(results[c]["spk"].astype(np.float32), 0.0)
        outs.append(                                # [NTOKC, 128, T, HC, BC]
            np.ascontiguousarray(np.transpose(spk, (0, 4, 2, 3, 1))).reshape(BL, T, H)
        )
    return np.concatenate(outs, axis=0)


def kernel(data, W, b, w1, b1, w2, b2):
    import sys
    if "/opt/trn_rl_repo" not in sys.path:
        sys.path.insert(0, "/opt/trn_rl_repo")
    from concourse.bass_utils import run_bass_kernel_spmd

    nc = _build()
    in_maps = _host_prep(data, W, b, w1, b1, w2, b2)
    res = run_bass_kernel_spmd(nc, in_maps, list(range(NCORES)))
    return _gather(res.results).astype(np.float32)
